# revision 21
# baseline (speedup 1.0000x reference)
"""Causal self-attention block (LN -> QKV -> causal attention -> out-proj)
on 8 Trainium2 NeuronCores.

Sharding: core = 2*batch + head_group. Each core handles one batch element
(S=2048 tokens) and 8 of the 16 heads (tensor-parallel split of w_qkv along
the head axis and w_out along its input dim). The two partial outputs per
batch are summed on the host (the all-reduce of the sharding hint).

Device kernel layout strategy (per core):
  - LayerNorm stats on DVE; rstd computed as exp(-0.5*ln(var+eps)) on the
    Activation engine so the whole kernel needs only ONE act table
    (natural_log_exp: ln+exp) -- no table thrash between LN and softmax.
  - Normalized x is cast to bf16 and transposed by the DMA xbar
    (dma_start_transpose), freeing the PE of all transposes.
  - QKV projection computes q^T/k^T in [head_dim, s] layout directly and V in
    natural [s, head_dim] layout, so causal attention needs no further
    transposes: scores are computed transposed, ST[k, q] = k . q, softmax'd
    along the partition-free axis via exp + a ones-column appended to V
    (the PV matmul then yields both y^T and the softmax row-sums).
  - Emission order interleaves QKV projection / LN of later blocks / output
    projection INTO the attention superblock loop so the in-order PE stream
    always has matmul work while the Activation engine runs softmax exps.
  - Softmax normalization: row-sums -> reciprocal (DVE) -> broadcast across
    partitions on the Pool engine (partition_broadcast) -> DVE multiply.
  - Output projection streams straight from PSUM to DRAM via DMA.
  - ln_scale/ln_bias/b_qkv/softmax-scale are folded into weights on host.
"""

import os

# the device path runs through jax's axon PJRT plugin; make sure a
# pre-set JAX_PLATFORMS doesn't hide it (unset = all plugins load)
_jp = os.environ.get("JAX_PLATFORMS")
if _jp and "axon" not in _jp:
    os.environ["JAX_PLATFORMS"] = f"axon,{_jp}"

import ml_dtypes
import numpy as np

import concourse.bass as bass
import concourse.mybir as mybir
import concourse.tile as tile
from concourse import bacc
from concourse.bass_utils import run_bass_kernel_spmd
from concourse.masks import make_identity

B, S, D, H, HD = 4, 2048, 1024, 16, 64
HL = H // 2          # heads per core (local)
NCH = D // 128       # 8 contraction chunks
NSB = S // 128       # 16 s-blocks
NQS = S // 512       # 4 q-superblocks
NEG = -1.0e38
LN_EPS = 1e-6

f32 = mybir.dt.float32
f32r = mybir.dt.float32r
bf16 = mybir.dt.bfloat16

_cache = {}

DEBUG_DUMPS = False


def build_program():
    nc = bacc.Bacc()

    x_d = nc.declare_dram_parameter("x", [S, D], f32, isOutput=False)
    wqk_d = nc.declare_dram_parameter("wqk", [NCH, 128, 1024], bf16, isOutput=False)
    wv_d = nc.declare_dram_parameter("wv", [NCH, 128, 512], bf16, isOutput=False)
    bqk_d = nc.declare_dram_parameter("bqk", [128, 2, 4], f32, isOutput=False)
    bv1_d = nc.declare_dram_parameter("bv1", [1, 512], f32r, isOutput=False)
    vones_d = nc.declare_dram_parameter("vones", [1, 128], f32r, isOutput=False)
    wout_d = nc.declare_dram_parameter("wout", [4, 128, 1024], bf16, isOutput=False)
    out_d = nc.declare_dram_parameter("out", [S, D], f32, isOutput=True)
    if DEBUG_DUMPS:
        dbg = {
            "xnT": nc.declare_dram_parameter("d_xnT", [128, NCH, S], bf16, isOutput=True),
            "qT": nc.declare_dram_parameter("d_qT", [128, 4, S], bf16, isOutput=True),
            "kT": nc.declare_dram_parameter("d_kT", [128, 4, S], bf16, isOutput=True),
            "vpp": nc.declare_dram_parameter("d_vpp", [128, NSB, HL, HD + 1], bf16, isOutput=True),
            "ytall": nc.declare_dram_parameter("d_ytall", [128, 4, S], bf16, isOutput=True),
            "xh": nc.declare_dram_parameter("d_xh", [4, 128, D], bf16, isOutput=True),
            "rstd": nc.declare_dram_parameter("d_rstd", [4, 128, 1], f32, isOutput=True),
        }

    with tile.TileContext(nc, pool_alloc_mode="queue") as tc:
        with (
            tc.tile_pool(name="singles", bufs=1) as singles,
            tc.tile_pool(name="qkT", bufs=1) as qkTp,
            tc.tile_pool(name="vpool", bufs=1) as vpool,
            tc.tile_pool(name="xnTp", bufs=1) as xnTp,
            tc.tile_pool(name="ytallp", bufs=1) as ytallp,
            tc.tile_pool(name="xpool", bufs=3) as xpool,
            tc.tile_pool(name="spool", bufs=8) as spool,
            tc.tile_pool(name="ptp", bufs=5) as ptp,
            tc.tile_pool(name="epi", bufs=2) as epip,
            tc.tile_pool(name="dstage", bufs=4, space="DRAM") as dstage,
            tc.tile_pool(name="pscm", bufs=1, space="PSUM") as pscm,
        ):
            # chain every DMA so the single in-order DMA FIFO processes in
            # exactly emission order: xbar-mode transitions (copy<->transpose
            # serialize on full drain) then only hit group boundaries we chose
            dma_chain = [None]

            def chained_dma(inst):
                if dma_chain[0] is not None:
                    bass._add_dep_helper(
                        inst.ins, dma_chain[0].ins, sync=False,
                        reason="dma fifo order",
                    )
                dma_chain[0] = inst
                return inst

            def cdma(out, in_):
                return chained_dma(nc.sync.dma_start(out=out, in_=in_))

            def cdma_t(out, in_):
                return chained_dma(nc.sync.dma_start_transpose(out=out, in_=in_))

            # ---- constants ----
            identb = singles.tile([128, 128], bf16)
            make_identity(nc, identb)
            maskTb = singles.tile([128, 128], bf16)
            nc.gpsimd.memset(maskTb, 0.0)
            nc.gpsimd.affine_select(
                out=maskTb, in_=maskTb,
                compare_op=mybir.AluOpType.is_ge,
                fill=NEG, base=0,
                pattern=[[1, 128]], channel_multiplier=-1,
            )
            eps_t = singles.tile([128, 1], f32)
            nc.vector.memset(eps_t, LN_EPS)
            bqk_t = singles.tile([128, 2, 4], f32)
            bv1_t = singles.tile([1, 512], f32r)
            vones_t = singles.tile([1, 128], f32r)

            # ---- weights (all resident in SBUF; bf16) ----
            # the sim models ONE in-order DMA FIFO with head-of-line
            # blocking, so DMA emission order is scheduling: independent
            # loads (x blocks, weight chunks) go first, dependent DMAs
            # (xbar transposes of xh) are emitted right after their
            # producers so the queue never stalls long on them
            wqk_t = singles.tile([128, NCH, 1024], bf16)
            wv_t = singles.tile([128, NCH, 512], bf16)
            wout_t = singles.tile([128, 4, 1024], bf16)

            def load_wqk(fb):
                cdma(
                    out=wqk_t[:, :, fb * 128 : (fb + 1) * 128],
                    in_=wqk_d[:, :, fb * 128 : (fb + 1) * 128].rearrange(
                        "c d f -> d c f"
                    ),
                )

            # ---- persistent activations ----
            qT = qkTp.tile([128, 4, S], bf16)   # [pair-row, pair, s]
            kT = qkTp.tile([128, 4, S], bf16)
            # V'' [s-row, s-block, head, 65] (col 64 = ones)
            vpp = vpool.tile([128, NSB, HL, HD + 1], bf16)
            nc.gpsimd.memset(vpp[:, :, :, HD : HD + 1], 1.0)
            xnT = xnTp.tile([128, NCH, S], bf16)
            ytall = ytallp.tile([128, 4, S], bf16)  # [pair-row, pair, s]

            # ================= per-block LN + DMA-xbar transpose ==========
            # rstd = exp(-0.5 * ln(var + eps)).  Ln and Exp live in different
            # greedy act tables, so the Ln/Exp ops are BATCHED per 4-block
            # group to bound table reloads (2 per group).  Every Activation-
            # engine op is chained with a nosync dep so the tile scheduler
            # cannot interleave Ln/Exp runs (which would thrash tables).
            ln_state = {}
            act_chain = [None]

            def chained_act(**kw):
                inst = nc.scalar.activation(**kw)
                if act_chain[0] is not None:
                    bass._add_dep_helper(
                        inst.ins, act_chain[0].ins, sync=False,
                        reason="act table batching order",
                    )
                act_chain[0] = inst
                return inst

            def emit_ln_load(i):
                x_t = xpool.tile([128, D], f32, tag="x", bufs=6, name=f"x_{i}")
                cdma(out=x_t, in_=x_d[i * 128 : (i + 1) * 128, :])
                ln_state[i] = (x_t,)

            def emit_ln_stats(i):
                if i not in ln_state:
                    emit_ln_load(i)
                (x_t,) = ln_state[i]
                stats = spool.tile([128, 2, 6], f32, tag="stats")
                nc.vector.bn_stats(out=stats[:, 0, :], in_=x_t[:, 0:512])
                nc.vector.bn_stats(out=stats[:, 1, :], in_=x_t[:, 512:1024])
                mv = spool.tile([128, 2], f32, tag="mv", name=f"mv_{i}")
                nc.vector.bn_aggr(out=mv, in_=stats)
                ln_state[i] = (x_t, mv)

            def emit_ln_acts(blocks):
                lnvs = {}
                for i in blocks:
                    lnv = spool.tile([128, 1], f32, tag="lnv",
                                     name=f"lnv_{i}")
                    chained_act(
                        out=lnv, in_=ln_state[i][1][:, 1:2],
                        func=mybir.ActivationFunctionType.Ln,
                        bias=eps_t, scale=1.0,
                    )
                    lnvs[i] = lnv
                for i in blocks:
                    rstd = spool.tile([128, 1], f32, tag="rstd",
                                      name=f"rstd_{i}")
                    chained_act(
                        out=rstd, in_=lnvs[i],
                        func=mybir.ActivationFunctionType.Exp,
                        bias=0.0, scale=-0.5,
                    )
                    ln_state[i] = ln_state[i] + (rstd,)

            def emit_ln_apply(i):
                x_t, mv, rstd = ln_state.pop(i)
                xh = xpool.tile([128, D], bf16, tag="xh", bufs=6,
                                name=f"xh_{i}")
                nc.vector.tensor_scalar(
                    out=xh, in0=x_t,
                    scalar1=mv[:, 0:1], scalar2=rstd,
                    op0=mybir.AluOpType.subtract, op1=mybir.AluOpType.mult,
                )
                # stage through DRAM: the executor's DmaTransposeAnt does not
                # reliably honor engine-write semaphores, but DMACopy does and
                # the DMA FIFO is in-order, so copy-then-transpose is safe
                xhd = dstage.tile([128, D], bf16, tag="xhd", name=f"xhd_{i}")
                cdma(out=xhd, in_=xh)
                cdma_t(
                    out=xnT[:, :, i * 128 : (i + 1) * 128], in_=xhd
                )

            def emit_ln_group(blocks):
                for i in blocks:
                    emit_ln_stats(i)
                emit_ln_acts(blocks)
                for i in blocks:
                    emit_ln_apply(i)

            # ================= QKV projection pieces ======================
            def emit_qk(p, sb):
                for t, dest in ((0, qT), (1, kT)):
                    fb = t * 4 + p
                    ps = pscm.tile([128, 512], f32, tag="misc", bufs=2,
                                   name=f"qkps_{t}_{p}_{sb}")
                    for c in range(NCH):
                        nc.tensor.matmul(
                            ps,
                            wqk_t[:, c, fb * 128 : (fb + 1) * 128],
                            xnT[:, c, sb * 512 : (sb + 1) * 512],
                            start=(c == 0),
                            stop=(c == NCH - 1),
                        )
                    nc.vector.tensor_scalar_add(
                        out=dest[:, p, sb * 512 : (sb + 1) * 512],
                        in0=ps,
                        scalar1=bqk_t[:, t, p : p + 1],
                    )

            def emit_v(j):
                psv = pscm.tile([128, 512], f32, tag="misc", bufs=2,
                                name=f"psv_{j}")
                for c in range(NCH):
                    nc.tensor.matmul(
                        psv,
                        xnT[:, c, j * 128 : (j + 1) * 128],
                        wv_t[:, c, :],
                        start=(c == 0),
                        stop=False,
                    )
                # += ones[s] x bv  (rank-1 bias update)
                nc.tensor.matmul(psv, vones_t, bv1_t, start=False, stop=True)
                nc.vector.tensor_copy(
                    vpp[:, j, :, 0:HD],
                    psv.rearrange("p (h v) -> p h v", v=HD),
                )

            # ================= attention unit (sb, p) =====================
            def emit_attn_unit(sb, p):
                q0 = sb * 512
                jmax = 4 * sb + 3
                yts = [
                    pscm.tile([HD + 1, 512], f32, tag="yt", bufs=2,
                              name=f"yt_{sb}_{p}_{hf}")
                    for hf in range(2)
                ]
                sts = {}
                pts = {}

                def emit_scores(j):
                    r = max(0, j - 4 * sb)
                    diag = j >= 4 * sb
                    L = 512 - 128 * r
                    st = pscm.tile([128, 1024], f32, tag="st", bufs=2,
                                   name=f"st_{sb}_{p}_{j}")
                    for hf in range(2):
                        rows = slice(hf * HD, (hf + 1) * HD)
                        # hf0 packs left in bank 0; hf1 must stay bank-aligned
                        # at 512 (matmul outputs cannot cross a PSUM bank)
                        lo = hf * 512
                        nc.tensor.matmul(
                            st[:, lo : lo + L],
                            kT[rows, p, j * 128 : (j + 1) * 128],
                            qT[rows, p, q0 + r * 128 : q0 + 512],
                            start=True, stop=not diag,
                        )
                    if diag:
                        # causal mask folded in on the PE:
                        # st[diag] += I.T @ maskT
                        for hf in range(2):
                            nc.tensor.matmul(
                                st[:, hf * 512 : hf * 512 + 128],
                                identb,
                                maskTb,
                                start=False, stop=True,
                            )
                    sts[j] = (st, L)

                def emit_exp(j):
                    st, L = sts.pop(j)
                    pt = ptp.tile([128, 1024], bf16, tag="pt")
                    # one wide exp across both heads (for r>0 the [L:512)
                    # strip is unread garbage)
                    chained_act(
                        out=pt[:, 0 : 512 + L],
                        in_=st[:, 0 : 512 + L],
                        func=mybir.ActivationFunctionType.Exp,
                    )
                    pts[j] = (pt, L)

                def emit_pv(j):
                    pt, L = pts.pop(j)
                    r = (512 - L) // 128
                    for hf in range(2):
                        nc.tensor.matmul(
                            yts[hf][:, r * 128 : 512],
                            vpp[:, j, 2 * p + hf, :],
                            pt[:, hf * 512 : hf * 512 + L],
                            start=(j == 0),
                            stop=(j == jmax),
                        )

                # software pipeline: scores(j+1) issued before pv(j) so the
                # in-order PE never head-of-line blocks on exp(j)
                emit_scores(0)
                for j in range(jmax + 1):
                    if j + 1 <= jmax:
                        emit_scores(j + 1)
                    emit_exp(j)
                    emit_pv(j)

                # softmax normalization epilogue (row 64 of yts = sums)
                for hf in range(2):
                    rows = slice(hf * HD, (hf + 1) * HD)
                    yt = yts[hf]
                    sinv = epip.tile([1, 512], f32, tag="sinv")
                    nc.vector.reciprocal(out=sinv, in_=yt[HD : HD + 1, :])
                    binv = epip.tile([HD, 512], f32, tag="binv")
                    nc.gpsimd.partition_broadcast(binv, sinv, channels=HD)
                    nc.vector.tensor_mul(
                        out=ytall[rows, p, q0 : q0 + 512],
                        in0=yt[0:HD, :],
                        in1=binv,
                    )

            # ================= output projection ==========================
            def emit_outproj(i):
                y_t = xpool.tile([128, 1024], f32, tag="y", name=f"y_{i}")
                for nh in range(2):
                    pso = pscm.tile([128, 512], f32, tag="misc", bufs=2,
                                    name=f"pso_{i}_{nh}")
                    for c in range(4):
                        nc.tensor.matmul(
                            pso,
                            ytall[:, c, i * 128 : (i + 1) * 128],
                            wout_t[:, c, nh * 512 : (nh + 1) * 512],
                            start=(c == 0),
                            stop=(c == 3),
                        )
                    nc.vector.tensor_copy(
                        y_t[:, nh * 512 : (nh + 1) * 512], pso
                    )
                cdma(
                    out=out_d[i * 128 : (i + 1) * 128, :], in_=y_t
                )

            # ================= emission schedule ==========================
            # DMA queue order: independent loads first (x blocks 0..7 and
            # the weight chunks needed earliest), dependent transposes after
            # their producers
            for i in range(4):
                emit_ln_load(i)
            load_wqk(0)
            load_wqk(4)
            for c in range(NCH):
                cdma(out=wv_t[:, c, :], in_=wv_d[c, :, :])
            cdma(out=bqk_t, in_=bqk_d[:, :, :])
            cdma(out=bv1_t, in_=bv1_d[:, :])
            cdma(out=vones_t, in_=vones_d[:, :])
            emit_ln_group(range(0, 4))
            for i in range(4, 8):
                emit_ln_load(i)
            emit_ln_group(range(4, 8))
            for fb in (1, 5, 2, 6, 3, 7):
                load_wqk(fb)
            for j in range(4):
                emit_v(j)
            emit_qk(0, 0)
            for c in range(4):
                cdma(out=wout_t[:, c, :], in_=wout_d[c, :, :])

            for sb in range(NQS):
                for p in range(4):
                    if sb == 0 and p >= 1:
                        emit_qk(p, 0)
                    emit_attn_unit(sb, p)
                    if sb < 3:
                        emit_qk(p, sb + 1)
                        emit_v(4 * (sb + 1) + p)
                    if sb <= 1:
                        # stats spread across units; the Act-table-sensitive
                        # ln/exp ops + xh batched after the last unit
                        emit_ln_stats(8 + 4 * sb + p)
                        if p == 3:
                            blocks = range(8 + 4 * sb, 12 + 4 * sb)
                            emit_ln_acts(blocks)
                            for i in blocks:
                                emit_ln_apply(i)
                    if sb >= 1:
                        emit_outproj(4 * (sb - 1) + p)
            for p in range(4):
                emit_outproj(12 + p)

            if DEBUG_DUMPS:
                cdma(out=dbg["xnT"][:, :, :], in_=xnT)
                cdma(out=dbg["qT"][:, :, :], in_=qT)
                cdma(out=dbg["kT"][:, :, :], in_=kT)
                cdma(out=dbg["vpp"][:, :, :, :], in_=vpp)
                cdma(out=dbg["ytall"][:, :, :], in_=ytall)

    nc.finalize()
    return nc


def _prep_core_inputs(x, ln_scale, ln_bias, w_qkv, b_qkv, w_out):
    """Host-side folding + per-core input maps."""
    scale = np.float32(HD ** -0.5)
    # qkv = xn@W + b_qkv, xn = z*ln_scale + ln_bias  =>  z @ (ln_scale*W) + (ln_bias@W + b_qkv)
    b_eff = b_qkv + np.einsum(
        "d,dhf->hf", ln_bias.astype(np.float64), w_qkv.astype(np.float64)
    ).astype(np.float32)
    w_eff = ln_scale[:, None, None] * w_qkv
    wq = w_eff[:, :, 0:64] * scale
    wk = w_eff[:, :, 64:128]
    wv = w_eff[:, :, 128:192]
    bq = b_eff[:, 0:64] * scale
    bk = b_eff[:, 64:128]
    bv = b_eff[:, 128:192]

    in_maps = []
    for core in range(8):
        b, g = core // 2, core % 2
        hsel = slice(g * HL, (g + 1) * HL)
        # [D, 4 pairs, 128] with head 2p in rows 0:64, head 2p+1 in 64:128
        qp = wq[:, hsel].reshape(D, 4, 128)
        kp = wk[:, hsel].reshape(D, 4, 128)
        wqk = np.concatenate(
            [qp.reshape(D, 512), kp.reshape(D, 512)], axis=1
        ).reshape(NCH, 128, 1024).astype(ml_dtypes.bfloat16)
        wv_g = np.ascontiguousarray(wv[:, hsel].reshape(D, 512)).reshape(
            NCH, 128, 512
        ).astype(ml_dtypes.bfloat16)
        bq_p = bq[hsel].reshape(4, 128)
        bk_p = bk[hsel].reshape(4, 128)
        bqk = np.ascontiguousarray(
            np.stack([bq_p, bk_p], axis=0).transpose(2, 0, 1)
        )
        bv1 = np.ascontiguousarray(bv[hsel].reshape(1, 512))
        wout = np.ascontiguousarray(
            w_out[g * 512 : (g + 1) * 512, :].reshape(4, 128, 1024)
        ).astype(ml_dtypes.bfloat16)
        in_maps.append(
            {
                "x": np.ascontiguousarray(x[b]),
                "wqk": np.ascontiguousarray(wqk),
                "wv": wv_g,
                "bqk": bqk,
                "bv1": bv1,
                "vones": np.ones((1, 128), np.float32),
                "wout": wout,
            }
        )
    return in_maps


def kernel(x, mask, ln_scale, ln_bias, w_qkv, b_qkv, w_out, b_out, **run_kwargs):
    x = np.asarray(x, np.float32)
    ln_scale = np.asarray(ln_scale, np.float32)
    ln_bias = np.asarray(ln_bias, np.float32)
    w_qkv = np.asarray(w_qkv, np.float32)
    b_qkv = np.asarray(b_qkv, np.float32)
    w_out = np.asarray(w_out, np.float32)
    b_out = np.asarray(b_out, np.float32)
    if "nc" not in _cache:
        _cache["nc"] = build_program()
    nc = _cache["nc"]
    in_maps = _prep_core_inputs(x, ln_scale, ln_bias, w_qkv, b_qkv, w_out)
    res = run_bass_kernel_spmd(nc, in_maps, list(range(8)), **run_kwargs)
    _cache["last_result"] = res
    out = np.empty((B, S, D), np.float32)
    for b in range(B):
        out[b] = res.results[2 * b]["out"] + res.results[2 * b + 1]["out"]
    out += np.asarray(b_out)[None, None, :]
    return out


# revision 29
# speedup vs baseline: 1.2969x; 1.2969x over previous
"""Causal self-attention block (LN -> QKV -> causal attention -> out-proj)
on 8 Trainium2 NeuronCores.

Sharding: core = 2*batch + head_group. Each core handles one batch element
(S=2048 tokens) and 8 of the 16 heads (tensor-parallel split of w_qkv along
the head axis and w_out along its input dim). The two partial outputs per
batch are summed on the host (the all-reduce of the sharding hint).

Device kernel layout strategy (per core):
  - LayerNorm stats on DVE; rstd computed as exp(-0.5*ln(var+eps)) on the
    Activation engine so the whole kernel needs only ONE act table
    (natural_log_exp: ln+exp) -- no table thrash between LN and softmax.
  - Normalized x is cast to bf16 and transposed by the DMA xbar
    (dma_start_transpose), freeing the PE of all transposes.
  - QKV projection computes q^T/k^T in [head_dim, s] layout directly and V in
    natural [s, head_dim] layout, so causal attention needs no further
    transposes: scores are computed transposed, ST[k, q] = k . q, softmax'd
    along the partition-free axis via exp + a ones-column appended to V
    (the PV matmul then yields both y^T and the softmax row-sums).
  - Emission order interleaves QKV projection / LN of later blocks / output
    projection INTO the attention superblock loop so the in-order PE stream
    always has matmul work while the Activation engine runs softmax exps.
  - Softmax normalization: row-sums -> reciprocal (DVE) -> broadcast across
    partitions on the Pool engine (partition_broadcast) -> DVE multiply.
  - Output projection streams straight from PSUM to DRAM via DMA.
  - ln_scale/ln_bias/b_qkv/softmax-scale are folded into weights on host.
"""

import os

# the device path runs through jax's axon PJRT plugin; make sure a
# pre-set JAX_PLATFORMS doesn't hide it (unset = all plugins load)
_jp = os.environ.get("JAX_PLATFORMS")
if _jp and "axon" not in _jp:
    os.environ["JAX_PLATFORMS"] = f"axon,{_jp}"

import ml_dtypes
import numpy as np

import concourse.bass as bass
import concourse.mybir as mybir
import concourse.tile as tile
from concourse import bacc
from concourse.bass_utils import run_bass_kernel_spmd
from concourse.masks import make_identity

B, S, D, H, HD = 4, 2048, 1024, 16, 64
HL = H // 2          # heads per core (local)
NCH = D // 128       # 8 contraction chunks
NSB = S // 128       # 16 s-blocks
NQS = S // 512       # 4 q-superblocks
NEG = -1.0e38
LN_EPS = 1e-6

f32 = mybir.dt.float32
f32r = mybir.dt.float32r
bf16 = mybir.dt.bfloat16

_cache = {}

DEBUG_DUMPS = False


def build_program():
    nc = bacc.Bacc()

    x_d = nc.declare_dram_parameter("x", [S, D], f32, isOutput=False)
    wqk_d = nc.declare_dram_parameter("wqk", [NCH, 128, 1024], bf16, isOutput=False)
    wv_d = nc.declare_dram_parameter("wv", [NCH, 128, 512], bf16, isOutput=False)
    bqk_d = nc.declare_dram_parameter("bqk", [128, 2, 4], f32, isOutput=False)
    bv1_d = nc.declare_dram_parameter("bv1", [1, 512], f32r, isOutput=False)
    vones_d = nc.declare_dram_parameter("vones", [1, 128], f32r, isOutput=False)
    wout_d = nc.declare_dram_parameter("wout", [4, 128, 1024], bf16, isOutput=False)
    out_d = nc.declare_dram_parameter("out", [S, D], f32, isOutput=True)
    if DEBUG_DUMPS:
        dbg = {
            "xnT": nc.declare_dram_parameter("d_xnT", [128, NCH, S], bf16, isOutput=True),
            "qT": nc.declare_dram_parameter("d_qT", [128, 4, S], bf16, isOutput=True),
            "kT": nc.declare_dram_parameter("d_kT", [128, 4, S], bf16, isOutput=True),
            "vpp": nc.declare_dram_parameter("d_vpp", [128, NSB, HL, HD + 1], bf16, isOutput=True),
            "ytall": nc.declare_dram_parameter("d_ytall", [128, 4, S], bf16, isOutput=True),
            "xh": nc.declare_dram_parameter("d_xh", [4, 128, D], bf16, isOutput=True),
            "rstd": nc.declare_dram_parameter("d_rstd", [4, 128, 1], f32, isOutput=True),
        }

    with tile.TileContext(nc, pool_alloc_mode="queue") as tc:
        with (
            tc.tile_pool(name="singles", bufs=1) as singles,
            tc.tile_pool(name="qkT", bufs=1) as qkTp,
            tc.tile_pool(name="vpool", bufs=1) as vpool,
            tc.tile_pool(name="xnTp", bufs=1) as xnTp,
            tc.tile_pool(name="ytallp", bufs=1) as ytallp,
            tc.tile_pool(name="xpool", bufs=3) as xpool,
            tc.tile_pool(name="spool", bufs=8) as spool,
            tc.tile_pool(name="ptp", bufs=4) as ptp,
            tc.tile_pool(name="epi", bufs=2) as epip,
            tc.tile_pool(name="dstage", bufs=4, space="DRAM") as dstage,
            tc.tile_pool(name="pscm", bufs=1, space="PSUM") as pscm,
        ):
            # chain every DMA so the single in-order DMA FIFO processes in
            # exactly emission order: xbar-mode transitions (copy<->transpose
            # serialize on full drain) then only hit group boundaries we chose
            dma_chain = [None]

            def chained_dma(inst):
                if dma_chain[0] is not None:
                    bass._add_dep_helper(
                        inst.ins, dma_chain[0].ins, sync=False,
                        reason="dma fifo order",
                    )
                dma_chain[0] = inst
                return inst

            def cdma(out, in_):
                return chained_dma(nc.sync.dma_start(out=out, in_=in_))

            def cdma_t(out, in_):
                return chained_dma(nc.sync.dma_start_transpose(out=out, in_=in_))

            # ---- constants ----
            identb = singles.tile([128, 128], bf16)
            make_identity(nc, identb)
            identf = singles.tile([128, 128], f32)
            make_identity(nc, identf)
            maskTb = singles.tile([128, 128], bf16)
            nc.gpsimd.memset(maskTb, 0.0)
            nc.gpsimd.affine_select(
                out=maskTb, in_=maskTb,
                compare_op=mybir.AluOpType.is_ge,
                fill=NEG, base=0,
                pattern=[[1, 128]], channel_multiplier=-1,
            )
            eps_t = singles.tile([128, 1], f32)
            nc.vector.memset(eps_t, LN_EPS)
            bqk_t = singles.tile([128, 2, 4], f32)
            bv1_t = singles.tile([1, 512], f32r)
            vones_t = singles.tile([1, 128], f32r)

            # ---- weights (all resident in SBUF; bf16) ----
            # the sim models ONE in-order DMA FIFO with head-of-line
            # blocking, so DMA emission order is scheduling: independent
            # loads (x blocks, weight chunks) go first, dependent DMAs
            # (xbar transposes of xh) are emitted right after their
            # producers so the queue never stalls long on them
            wqk_t = singles.tile([128, NCH, 1024], bf16)
            wv_t = singles.tile([128, NCH, 512], bf16)
            wout_t = singles.tile([128, 4, 1024], bf16)

            def load_wqk(fb):
                cdma(
                    out=wqk_t[:, :, fb * 128 : (fb + 1) * 128],
                    in_=wqk_d[:, :, fb * 128 : (fb + 1) * 128].rearrange(
                        "c d f -> d c f"
                    ),
                )

            # ---- persistent activations ----
            qT = qkTp.tile([128, 4, S], bf16)   # [pair-row, pair, s]
            kT = qkTp.tile([128, 4, S], bf16)
            # V'' [s-row, s-block, head, 65] (col 64 = ones)
            vpp = vpool.tile([128, NSB, HL, HD + 1], bf16)
            nc.gpsimd.memset(vpp[:, :, :, HD : HD + 1], 1.0)
            xnT = xnTp.tile([128, NCH, S], bf16)
            ytall = ytallp.tile([128, 4, S], bf16)  # [pair-row, pair, s]

            # ================= per-block LN + DMA-xbar transpose ==========
            # rstd = exp(-0.5 * ln(var + eps)).  Ln and Exp live in different
            # greedy act tables, so the Ln/Exp ops are BATCHED per 4-block
            # group to bound table reloads (2 per group).  Every Activation-
            # engine op is chained with a nosync dep so the tile scheduler
            # cannot interleave Ln/Exp runs (which would thrash tables).
            ln_state = {}
            act_chain = [None]

            def chained_act(**kw):
                inst = nc.scalar.activation(**kw)
                if act_chain[0] is not None:
                    bass._add_dep_helper(
                        inst.ins, act_chain[0].ins, sync=False,
                        reason="act table batching order",
                    )
                act_chain[0] = inst
                return inst

            def emit_ln_load(i):
                x_t = xpool.tile([128, D], f32, tag="x", bufs=6, name=f"x_{i}")
                cdma(out=x_t, in_=x_d[i * 128 : (i + 1) * 128, :])
                ln_state[i] = (x_t,)

            def emit_ln_stats(i):
                if i not in ln_state:
                    emit_ln_load(i)
                (x_t,) = ln_state[i]
                stats = spool.tile([128, 2, 6], f32, tag="stats")
                nc.vector.bn_stats(out=stats[:, 0, :], in_=x_t[:, 0:512])
                nc.vector.bn_stats(out=stats[:, 1, :], in_=x_t[:, 512:1024])
                mv = spool.tile([128, 2], f32, tag="mv", name=f"mv_{i}")
                nc.vector.bn_aggr(out=mv, in_=stats)
                ln_state[i] = (x_t, mv)

            def emit_ln_acts(blocks):
                lnvs = {}
                for i in blocks:
                    lnv = spool.tile([128, 1], f32, tag="lnv",
                                     name=f"lnv_{i}")
                    chained_act(
                        out=lnv, in_=ln_state[i][1][:, 1:2],
                        func=mybir.ActivationFunctionType.Ln,
                        bias=eps_t, scale=1.0,
                    )
                    lnvs[i] = lnv
                for i in blocks:
                    rstd = spool.tile([128, 1], f32, tag="rstd",
                                      name=f"rstd_{i}")
                    chained_act(
                        out=rstd, in_=lnvs[i],
                        func=mybir.ActivationFunctionType.Exp,
                        bias=0.0, scale=-0.5,
                    )
                    ln_state[i] = ln_state[i] + (rstd,)

            def emit_ln_apply(i, pe=False):
                x_t, mv, rstd = ln_state.pop(i)
                if pe:
                    # PE-transpose path for the first blocks: the PE is idle
                    # during the prologue and this also warms its pstate
                    xh = xpool.tile([128, D], f32, tag="xhf", bufs=4,
                                    name=f"xhf_{i}")
                else:
                    xh = xpool.tile([128, D], bf16, tag="xh", bufs=3,
                                    name=f"xh_{i}")
                nc.vector.tensor_scalar(
                    out=xh, in0=x_t,
                    scalar1=mv[:, 0:1], scalar2=rstd,
                    op0=mybir.AluOpType.subtract, op1=mybir.AluOpType.mult,
                )
                if pe:
                    for g in range(2):
                        pst = pscm.tile([128, 4, 128], f32, tag="misc",
                                        bufs=2, name=f"pst_{i}_{g}")
                        for k in range(4):
                            c = 4 * g + k
                            nc.tensor.transpose(
                                pst[:, k, :],
                                xh[:, c * 128 : (c + 1) * 128],
                                identf,
                            )
                        chained_act(
                            out=xnT[:, 4 * g : 4 * g + 4,
                                    i * 128 : (i + 1) * 128],
                            in_=pst,
                            func=mybir.ActivationFunctionType.Copy,
                        )
                    return
                # stage xh through DRAM: the compiled DmaTransposeAnt path
                # does not reliably honor engine-write semaphores on its SBUF
                # source, but a DRAM source written by an ordinary DMACopy
                # (sem-correct) on the same in-order FIFO is safe
                xhd = dstage.tile([128, D], bf16, tag="xhd", name=f"xhd_{i}")
                cdma(out=xhd, in_=xh)
                ln_state[i] = xhd

            def emit_ln_transpose(i):
                xhd = ln_state.pop(i)
                cdma_t(
                    out=xnT[:, :, i * 128 : (i + 1) * 128], in_=xhd
                )

            def emit_ln_group(blocks, pe=False):
                for i in blocks:
                    emit_ln_stats(i)
                emit_ln_acts(blocks)
                for i in blocks:
                    emit_ln_apply(i, pe=pe)
                if not pe:
                    for i in blocks:
                        emit_ln_transpose(i)

            # ================= QKV projection pieces ======================
            def emit_qk(p, sb, on_act=False):
                for t, dest in ((0, qT), (1, kT)):
                    fb = t * 4 + p
                    ps = pscm.tile([128, 512], f32, tag="misc", bufs=2,
                                   name=f"qkps_{t}_{p}_{sb}")
                    for c in range(NCH):
                        nc.tensor.matmul(
                            ps,
                            wqk_t[:, c, fb * 128 : (fb + 1) * 128],
                            xnT[:, c, sb * 512 : (sb + 1) * 512],
                            start=(c == 0),
                            stop=(c == NCH - 1),
                        )
                    if on_act:
                        # bias-add as Identity activation: Identity is in
                        # every act table, and the Act engine idles while the
                        # DVE saturates in these stretches
                        chained_act(
                            out=dest[:, p, sb * 512 : (sb + 1) * 512],
                            in_=ps,
                            func=mybir.ActivationFunctionType.Identity,
                            bias=bqk_t[:, t, p : p + 1], scale=1.0,
                        )
                    else:
                        nc.vector.tensor_scalar_add(
                            out=dest[:, p, sb * 512 : (sb + 1) * 512],
                            in0=ps,
                            scalar1=bqk_t[:, t, p : p + 1],
                        )

            def emit_v(j):
                psv = pscm.tile([128, 512], f32, tag="misc", bufs=2,
                                name=f"psv_{j}")
                for c in range(NCH):
                    nc.tensor.matmul(
                        psv,
                        xnT[:, c, j * 128 : (j + 1) * 128],
                        wv_t[:, c, :],
                        start=(c == 0),
                        stop=False,
                    )
                # += ones[s] x bv  (rank-1 bias update)
                nc.tensor.matmul(psv, vones_t, bv1_t, start=False, stop=True)
                return psv

            def emit_v_copy(j, psv, on_act=False):
                if on_act:
                    chained_act(
                        out=vpp[:, j, :, 0:HD],
                        in_=psv.rearrange("p (h v) -> p h v", v=HD),
                        func=mybir.ActivationFunctionType.Copy,
                    )
                else:
                    nc.vector.tensor_copy(
                        vpp[:, j, :, 0:HD],
                        psv.rearrange("p (h v) -> p h v", v=HD),
                    )

            # ================= attention unit (sb, p) =====================
            def emit_attn_unit(sb, p):
                q0 = sb * 512
                jmax = 4 * sb + 3
                yts = [
                    pscm.tile([HD + 1, 512], f32, tag="yt", bufs=2,
                              name=f"yt_{sb}_{p}_{hf}")
                    for hf in range(2)
                ]
                sts = {}
                pts = {}

                def emit_scores(j):
                    r = max(0, j - 4 * sb)
                    diag = j >= 4 * sb
                    L = 512 - 128 * r
                    st = pscm.tile([128, 1024], f32, tag="st", bufs=2,
                                   name=f"st_{sb}_{p}_{j}")
                    for hf in range(2):
                        rows = slice(hf * HD, (hf + 1) * HD)
                        # hf0 packs left in bank 0; hf1 must stay bank-aligned
                        # at 512 (matmul outputs cannot cross a PSUM bank)
                        lo = hf * 512
                        nc.tensor.matmul(
                            st[:, lo : lo + L],
                            kT[rows, p, j * 128 : (j + 1) * 128],
                            qT[rows, p, q0 + r * 128 : q0 + 512],
                            start=True, stop=not diag,
                        )
                    if diag:
                        # causal mask folded in on the PE:
                        # st[diag] += I.T @ maskT
                        for hf in range(2):
                            nc.tensor.matmul(
                                st[:, hf * 512 : hf * 512 + 128],
                                identb,
                                maskTb,
                                start=False, stop=True,
                            )
                    sts[j] = (st, L)

                def emit_exp(j):
                    st, L = sts.pop(j)
                    pt = ptp.tile([128, 1024], bf16, tag="pt")
                    # one wide exp across both heads (for r>0 the [L:512)
                    # strip is unread garbage)
                    chained_act(
                        out=pt[:, 0 : 512 + L],
                        in_=st[:, 0 : 512 + L],
                        func=mybir.ActivationFunctionType.Exp,
                    )
                    pts[j] = (pt, L)

                def emit_pv(j):
                    pt, L = pts.pop(j)
                    r = (512 - L) // 128
                    for hf in range(2):
                        nc.tensor.matmul(
                            yts[hf][:, r * 128 : 512],
                            vpp[:, j, 2 * p + hf, :],
                            pt[:, hf * 512 : hf * 512 + L],
                            start=(j == 0),
                            stop=(j == jmax),
                        )

                # software pipeline: scores(j+1) issued before pv(j) so the
                # in-order PE never head-of-line blocks on exp(j)
                emit_scores(0)
                for j in range(jmax + 1):
                    if j + 1 <= jmax:
                        emit_scores(j + 1)
                    emit_exp(j)
                    emit_pv(j)
                return yts

            # softmax normalization epilogue (row 64 of yts = sums); emitted
            # AFTER the next filler work so the DVE drains the PSUM-freeing
            # bias-adds/copies first
            def emit_epilogue(sb, p, yts):
                q0 = sb * 512
                for hf in range(2):
                    rows = slice(hf * HD, (hf + 1) * HD)
                    yt = yts[hf]
                    sinv = epip.tile([1, 512], f32, tag="sinv")
                    nc.vector.reciprocal(out=sinv, in_=yt[HD : HD + 1, :])
                    binv = epip.tile([HD, 512], f32, tag="binv")
                    nc.gpsimd.partition_broadcast(binv, sinv, channels=HD)
                    nc.vector.tensor_mul(
                        out=ytall[rows, p, q0 : q0 + 512],
                        in0=yt[0:HD, :],
                        in1=binv,
                    )

            # ================= output projection ==========================
            def emit_outproj(i):
                y_t = xpool.tile([128, 1024], f32, tag="y", bufs=2, name=f"y_{i}")
                for nh in range(2):
                    pso = pscm.tile([128, 512], f32, tag="misc", bufs=2,
                                    name=f"pso_{i}_{nh}")
                    for c in range(4):
                        nc.tensor.matmul(
                            pso,
                            ytall[:, c, i * 128 : (i + 1) * 128],
                            wout_t[:, c, nh * 512 : (nh + 1) * 512],
                            start=(c == 0),
                            stop=(c == 3),
                        )
                    nc.vector.tensor_copy(
                        y_t[:, nh * 512 : (nh + 1) * 512], pso
                    )
                cdma(
                    out=out_d[i * 128 : (i + 1) * 128, :], in_=y_t
                )

            # ================= emission schedule ==========================
            # DMA queue order: independent loads first (x blocks 0..7 and
            # the weight chunks needed earliest), dependent transposes after
            # their producers
            for i in range(8):
                emit_ln_load(i)
            load_wqk(0)
            load_wqk(4)
            for c in range(NCH):
                cdma(out=wv_t[:, c, :], in_=wv_d[c, :, :])
            cdma(out=bqk_t, in_=bqk_d[:, :, :])
            cdma(out=bv1_t, in_=bv1_d[:, :])
            cdma(out=vones_t, in_=vones_d[:, :])
            emit_ln_group(range(0, 4), pe=True)
            emit_ln_group(range(4, 8))
            for fb in (1, 5, 2, 6, 3, 7):
                load_wqk(fb)
            for j in range(4):
                emit_v_copy(j, emit_v(j))
            emit_qk(0, 0)
            for c in range(4):
                cdma(out=wout_t[:, c, :], in_=wout_d[c, :, :])

            for sb in range(NQS):
                for p in range(4):
                    if sb == 0 and p >= 1:
                        emit_qk(p, 0)
                    yts = emit_attn_unit(sb, p)
                    if sb < 3:
                        emit_qk(p, sb + 1, on_act=(sb <= 1))
                        emit_v_copy(4 * (sb + 1) + p,
                                    emit_v(4 * (sb + 1) + p),
                                    on_act=(sb <= 1))
                    if sb <= 1:
                        # stats early in the superblock; the Act-table-
                        # sensitive ln/exp ops + xh + transposes batched so
                        # xnT for the NEXT projections lands with slack
                        base = 8 + 4 * sb
                        if p <= 1:
                            emit_ln_stats(base + 2 * p)
                            emit_ln_stats(base + 2 * p + 1)
                        elif p == 2:
                            blocks = range(base, base + 4)
                            emit_ln_acts(blocks)
                            for i in blocks:
                                emit_ln_apply(i)
                            for i in blocks:
                                emit_ln_transpose(i)
                    emit_epilogue(sb, p, yts)
                    if sb >= 1:
                        emit_outproj(4 * (sb - 1) + p)
            for p in range(4):
                emit_outproj(12 + p)

            if DEBUG_DUMPS:
                cdma(out=dbg["xnT"][:, :, :], in_=xnT)
                cdma(out=dbg["qT"][:, :, :], in_=qT)
                cdma(out=dbg["kT"][:, :, :], in_=kT)
                cdma(out=dbg["vpp"][:, :, :, :], in_=vpp)
                cdma(out=dbg["ytall"][:, :, :], in_=ytall)

    nc.finalize()
    return nc


def _prep_core_inputs(x, ln_scale, ln_bias, w_qkv, b_qkv, w_out):
    """Host-side folding + per-core input maps."""
    scale = np.float32(HD ** -0.5)
    # qkv = xn@W + b_qkv, xn = z*ln_scale + ln_bias  =>  z @ (ln_scale*W) + (ln_bias@W + b_qkv)
    b_eff = b_qkv + np.einsum(
        "d,dhf->hf", ln_bias.astype(np.float64), w_qkv.astype(np.float64)
    ).astype(np.float32)
    w_eff = ln_scale[:, None, None] * w_qkv
    wq = w_eff[:, :, 0:64] * scale
    wk = w_eff[:, :, 64:128]
    wv = w_eff[:, :, 128:192]
    bq = b_eff[:, 0:64] * scale
    bk = b_eff[:, 64:128]
    bv = b_eff[:, 128:192]

    in_maps = []
    for core in range(8):
        b, g = core // 2, core % 2
        hsel = slice(g * HL, (g + 1) * HL)
        # [D, 4 pairs, 128] with head 2p in rows 0:64, head 2p+1 in 64:128
        qp = wq[:, hsel].reshape(D, 4, 128)
        kp = wk[:, hsel].reshape(D, 4, 128)
        wqk = np.concatenate(
            [qp.reshape(D, 512), kp.reshape(D, 512)], axis=1
        ).reshape(NCH, 128, 1024).astype(ml_dtypes.bfloat16)
        wv_g = np.ascontiguousarray(wv[:, hsel].reshape(D, 512)).reshape(
            NCH, 128, 512
        ).astype(ml_dtypes.bfloat16)
        bq_p = bq[hsel].reshape(4, 128)
        bk_p = bk[hsel].reshape(4, 128)
        bqk = np.ascontiguousarray(
            np.stack([bq_p, bk_p], axis=0).transpose(2, 0, 1)
        )
        bv1 = np.ascontiguousarray(bv[hsel].reshape(1, 512))
        wout = np.ascontiguousarray(
            w_out[g * 512 : (g + 1) * 512, :].reshape(4, 128, 1024)
        ).astype(ml_dtypes.bfloat16)
        in_maps.append(
            {
                "x": np.ascontiguousarray(x[b]),
                "wqk": np.ascontiguousarray(wqk),
                "wv": wv_g,
                "bqk": bqk,
                "bv1": bv1,
                "vones": np.ones((1, 128), np.float32),
                "wout": wout,
            }
        )
    return in_maps


def kernel(x, mask, ln_scale, ln_bias, w_qkv, b_qkv, w_out, b_out, **run_kwargs):
    x = np.asarray(x, np.float32)
    ln_scale = np.asarray(ln_scale, np.float32)
    ln_bias = np.asarray(ln_bias, np.float32)
    w_qkv = np.asarray(w_qkv, np.float32)
    b_qkv = np.asarray(b_qkv, np.float32)
    w_out = np.asarray(w_out, np.float32)
    b_out = np.asarray(b_out, np.float32)
    if "nc" not in _cache:
        _cache["nc"] = build_program()
    nc = _cache["nc"]
    in_maps = _prep_core_inputs(x, ln_scale, ln_bias, w_qkv, b_qkv, w_out)
    res = run_bass_kernel_spmd(nc, in_maps, list(range(8)), **run_kwargs)
    _cache["last_result"] = res
    out = np.empty((B, S, D), np.float32)
    for b in range(B):
        out[b] = res.results[2 * b]["out"] + res.results[2 * b + 1]["out"]
    out += np.asarray(b_out)[None, None, :]
    return out


# revision 33
# speedup vs baseline: 1.3029x; 1.0046x over previous
"""Causal self-attention block (LN -> QKV -> causal attention -> out-proj)
on 8 Trainium2 NeuronCores.

Sharding: core = 2*batch + head_group. Each core handles one batch element
(S=2048 tokens) and 8 of the 16 heads (tensor-parallel split of w_qkv along
the head axis and w_out along its input dim). The two partial outputs per
batch are summed on the host (the all-reduce of the sharding hint).

Device kernel layout strategy (per core):
  - LayerNorm stats on DVE; rstd computed as exp(-0.5*ln(var+eps)) on the
    Activation engine so the whole kernel needs only ONE act table
    (natural_log_exp: ln+exp) -- no table thrash between LN and softmax.
  - Normalized x is cast to bf16 and transposed by the DMA xbar
    (dma_start_transpose), freeing the PE of all transposes.
  - QKV projection computes q^T/k^T in [head_dim, s] layout directly and V in
    natural [s, head_dim] layout, so causal attention needs no further
    transposes: scores are computed transposed, ST[k, q] = k . q, softmax'd
    along the partition-free axis via exp + a ones-column appended to V
    (the PV matmul then yields both y^T and the softmax row-sums).
  - Emission order interleaves QKV projection / LN of later blocks / output
    projection INTO the attention superblock loop so the in-order PE stream
    always has matmul work while the Activation engine runs softmax exps.
  - Softmax normalization: row-sums -> reciprocal (DVE) -> broadcast across
    partitions on the Pool engine (partition_broadcast) -> DVE multiply.
  - Output projection streams straight from PSUM to DRAM via DMA.
  - ln_scale/ln_bias/b_qkv/softmax-scale are folded into weights on host.
"""

import os

# the device path runs through jax's axon PJRT plugin; make sure a
# pre-set JAX_PLATFORMS doesn't hide it (unset = all plugins load)
_jp = os.environ.get("JAX_PLATFORMS")
if _jp and "axon" not in _jp:
    os.environ["JAX_PLATFORMS"] = f"axon,{_jp}"

import ml_dtypes
import numpy as np

import concourse.bass as bass
import concourse.mybir as mybir
import concourse.tile as tile
from concourse import bacc
from concourse.bass_utils import run_bass_kernel_spmd
from concourse.masks import make_identity

B, S, D, H, HD = 4, 2048, 1024, 16, 64
HL = H // 2          # heads per core (local)
NCH = D // 128       # 8 contraction chunks
NSB = S // 128       # 16 s-blocks
NQS = S // 512       # 4 q-superblocks
NEG = -1.0e38
LN_EPS = 1e-6

f32 = mybir.dt.float32
f32r = mybir.dt.float32r
bf16 = mybir.dt.bfloat16

_cache = {}

DEBUG_DUMPS = False


def build_program():
    nc = bacc.Bacc()

    x_d = nc.declare_dram_parameter("x", [S, D], f32, isOutput=False)
    wqk_d = nc.declare_dram_parameter("wqk", [NCH, 128, 1024], bf16, isOutput=False)
    wv_d = nc.declare_dram_parameter("wv", [NCH, 128, 512], bf16, isOutput=False)
    bqk_d = nc.declare_dram_parameter("bqk", [128, 2, 4], f32, isOutput=False)
    bv1_d = nc.declare_dram_parameter("bv1", [1, 512], f32r, isOutput=False)
    vones_d = nc.declare_dram_parameter("vones", [1, 128], f32r, isOutput=False)
    wout_d = nc.declare_dram_parameter("wout", [4, 128, 1024], bf16, isOutput=False)
    out_d = nc.declare_dram_parameter("out", [S, D], f32, isOutput=True)
    if DEBUG_DUMPS:
        dbg = {
            "xnT": nc.declare_dram_parameter("d_xnT", [128, NCH, S], bf16, isOutput=True),
            "qT": nc.declare_dram_parameter("d_qT", [128, 4, S], bf16, isOutput=True),
            "kT": nc.declare_dram_parameter("d_kT", [128, 4, S], bf16, isOutput=True),
            "vpp": nc.declare_dram_parameter("d_vpp", [128, NSB, HL, HD + 1], bf16, isOutput=True),
            "ytall": nc.declare_dram_parameter("d_ytall", [128, 4, S], bf16, isOutput=True),
            "xh": nc.declare_dram_parameter("d_xh", [4, 128, D], bf16, isOutput=True),
            "rstd": nc.declare_dram_parameter("d_rstd", [4, 128, 1], f32, isOutput=True),
        }

    with tile.TileContext(nc, pool_alloc_mode="queue") as tc:
        with (
            tc.tile_pool(name="singles", bufs=1) as singles,
            tc.tile_pool(name="qkT", bufs=1) as qkTp,
            tc.tile_pool(name="vpool", bufs=1) as vpool,
            tc.tile_pool(name="xnTp", bufs=1) as xnTp,
            tc.tile_pool(name="ytallp", bufs=1) as ytallp,
            tc.tile_pool(name="xpool", bufs=3) as xpool,
            tc.tile_pool(name="spool", bufs=8) as spool,
            tc.tile_pool(name="ptp", bufs=4) as ptp,
            tc.tile_pool(name="epi", bufs=2) as epip,
            tc.tile_pool(name="dstage", bufs=4, space="DRAM") as dstage,
            tc.tile_pool(name="pscm", bufs=1, space="PSUM") as pscm,
        ):
            # chain every DMA so the single in-order DMA FIFO processes in
            # exactly emission order: xbar-mode transitions (copy<->transpose
            # serialize on full drain) then only hit group boundaries we chose
            dma_chain = [None]

            def chained_dma(inst):
                if dma_chain[0] is not None:
                    bass._add_dep_helper(
                        inst.ins, dma_chain[0].ins, sync=False,
                        reason="dma fifo order",
                    )
                dma_chain[0] = inst
                return inst

            def cdma(out, in_):
                return chained_dma(nc.sync.dma_start(out=out, in_=in_))

            def cdma_t(out, in_):
                return chained_dma(nc.sync.dma_start_transpose(out=out, in_=in_))

            # ---- constants ----
            identb = singles.tile([128, 128], bf16)
            make_identity(nc, identb)
            identf = singles.tile([128, 128], f32)
            make_identity(nc, identf)
            maskTb = singles.tile([128, 128], bf16)
            nc.gpsimd.memset(maskTb, 0.0)
            nc.gpsimd.affine_select(
                out=maskTb, in_=maskTb,
                compare_op=mybir.AluOpType.is_ge,
                fill=NEG, base=0,
                pattern=[[1, 128]], channel_multiplier=-1,
            )
            eps_t = singles.tile([128, 1], f32)
            nc.vector.memset(eps_t, LN_EPS)
            bqk_t = singles.tile([128, 2, 4], f32)
            bv1_t = singles.tile([1, 512], f32r)
            vones_t = singles.tile([1, 128], f32r)

            # ---- weights (all resident in SBUF; bf16) ----
            # the sim models ONE in-order DMA FIFO with head-of-line
            # blocking, so DMA emission order is scheduling: independent
            # loads (x blocks, weight chunks) go first, dependent DMAs
            # (xbar transposes of xh) are emitted right after their
            # producers so the queue never stalls long on them
            wqk_t = singles.tile([128, NCH, 1024], bf16)
            wv_t = singles.tile([128, NCH, 512], bf16)
            wout_t = singles.tile([128, 4, 1024], bf16)

            def load_wqk(fb):
                cdma(
                    out=wqk_t[:, :, fb * 128 : (fb + 1) * 128],
                    in_=wqk_d[:, :, fb * 128 : (fb + 1) * 128].rearrange(
                        "c d f -> d c f"
                    ),
                )

            # ---- persistent activations ----
            qT = qkTp.tile([128, 4, S], bf16)   # [pair-row, pair, s]
            kT = qkTp.tile([128, 4, S], bf16)
            # V'' [s-row, s-block, head, 65] (col 64 = ones)
            vpp = vpool.tile([128, NSB, HL, HD + 1], bf16)
            nc.gpsimd.memset(vpp[:, :, :, HD : HD + 1], 1.0)
            xnT = xnTp.tile([128, NCH, S], bf16)
            ytall = ytallp.tile([128, 4, S], bf16)  # [pair-row, pair, s]

            # ================= per-block LN + DMA-xbar transpose ==========
            # rstd = exp(-0.5 * ln(var + eps)).  Ln and Exp live in different
            # greedy act tables, so the Ln/Exp ops are BATCHED per 4-block
            # group to bound table reloads (2 per group).  Every Activation-
            # engine op is chained with a nosync dep so the tile scheduler
            # cannot interleave Ln/Exp runs (which would thrash tables).
            ln_state = {}
            act_chain = [None]

            def chained_act(**kw):
                inst = nc.scalar.activation(**kw)
                if act_chain[0] is not None:
                    bass._add_dep_helper(
                        inst.ins, act_chain[0].ins, sync=False,
                        reason="act table batching order",
                    )
                act_chain[0] = inst
                return inst

            def emit_ln_load(i):
                x_t = xpool.tile([128, D], f32, tag="x", bufs=6, name=f"x_{i}")
                cdma(out=x_t, in_=x_d[i * 128 : (i + 1) * 128, :])
                ln_state[i] = (x_t,)

            def emit_ln_stats(i):
                if i not in ln_state:
                    emit_ln_load(i)
                (x_t,) = ln_state[i]
                stats = spool.tile([128, 2, 6], f32, tag="stats")
                nc.vector.bn_stats(out=stats[:, 0, :], in_=x_t[:, 0:512])
                nc.vector.bn_stats(out=stats[:, 1, :], in_=x_t[:, 512:1024])
                mv = spool.tile([128, 2], f32, tag="mv", name=f"mv_{i}")
                nc.vector.bn_aggr(out=mv, in_=stats)
                ln_state[i] = (x_t, mv)

            def emit_ln_acts(blocks):
                lnvs = {}
                for i in blocks:
                    lnv = spool.tile([128, 1], f32, tag="lnv",
                                     name=f"lnv_{i}")
                    chained_act(
                        out=lnv, in_=ln_state[i][1][:, 1:2],
                        func=mybir.ActivationFunctionType.Ln,
                        bias=eps_t, scale=1.0,
                    )
                    lnvs[i] = lnv
                for i in blocks:
                    rstd = spool.tile([128, 1], f32, tag="rstd",
                                      name=f"rstd_{i}")
                    chained_act(
                        out=rstd, in_=lnvs[i],
                        func=mybir.ActivationFunctionType.Exp,
                        bias=0.0, scale=-0.5,
                    )
                    ln_state[i] = ln_state[i] + (rstd,)

            def emit_ln_apply(i, pe=False):
                x_t, mv, rstd = ln_state.pop(i)
                if pe:
                    # PE-transpose path for the first blocks: the PE is idle
                    # during the prologue and this also warms its pstate
                    xh = xpool.tile([128, D], f32, tag="xhf", bufs=4,
                                    name=f"xhf_{i}")
                else:
                    xh = xpool.tile([128, D], bf16, tag="xh", bufs=3,
                                    name=f"xh_{i}")
                nc.vector.tensor_scalar(
                    out=xh, in0=x_t,
                    scalar1=mv[:, 0:1], scalar2=rstd,
                    op0=mybir.AluOpType.subtract, op1=mybir.AluOpType.mult,
                )
                if pe:
                    for g in range(2):
                        pst = pscm.tile([128, 4, 128], f32, tag="misc",
                                        bufs=2, name=f"pst_{i}_{g}")
                        for k in range(4):
                            c = 4 * g + k
                            nc.tensor.transpose(
                                pst[:, k, :],
                                xh[:, c * 128 : (c + 1) * 128],
                                identf,
                            )
                        chained_act(
                            out=xnT[:, 4 * g : 4 * g + 4,
                                    i * 128 : (i + 1) * 128],
                            in_=pst,
                            func=mybir.ActivationFunctionType.Copy,
                        )
                    return
                # stage xh through DRAM: the compiled DmaTransposeAnt path
                # does not reliably honor engine-write semaphores on its SBUF
                # source, but a DRAM source written by an ordinary DMACopy
                # (sem-correct) on the same in-order FIFO is safe
                xhd = dstage.tile([128, D], bf16, tag="xhd", name=f"xhd_{i}")
                cdma(out=xhd, in_=xh)
                ln_state[i] = xhd

            def emit_ln_transpose(i):
                xhd = ln_state.pop(i)
                cdma_t(
                    out=xnT[:, :, i * 128 : (i + 1) * 128], in_=xhd
                )

            def emit_ln_group(blocks, pe=False):
                for i in blocks:
                    emit_ln_stats(i)
                emit_ln_acts(blocks)
                for i in blocks:
                    emit_ln_apply(i, pe=pe)
                if not pe:
                    for i in blocks:
                        emit_ln_transpose(i)

            # ================= QKV projection pieces ======================
            def emit_qk(p, sb, on_act=False):
                for t, dest in ((0, qT), (1, kT)):
                    fb = t * 4 + p
                    ps = pscm.tile([128, 512], f32, tag="misc", bufs=2,
                                   name=f"qkps_{t}_{p}_{sb}")
                    for c in range(NCH):
                        nc.tensor.matmul(
                            ps,
                            wqk_t[:, c, fb * 128 : (fb + 1) * 128],
                            xnT[:, c, sb * 512 : (sb + 1) * 512],
                            start=(c == 0),
                            stop=(c == NCH - 1),
                        )
                    if on_act:
                        # bias-add as Identity activation: Identity is in
                        # every act table, and the Act engine idles while the
                        # DVE saturates in these stretches
                        chained_act(
                            out=dest[:, p, sb * 512 : (sb + 1) * 512],
                            in_=ps,
                            func=mybir.ActivationFunctionType.Identity,
                            bias=bqk_t[:, t, p : p + 1], scale=1.0,
                        )
                    else:
                        nc.vector.tensor_scalar_add(
                            out=dest[:, p, sb * 512 : (sb + 1) * 512],
                            in0=ps,
                            scalar1=bqk_t[:, t, p : p + 1],
                        )

            def emit_v(j):
                psv = pscm.tile([128, 512], f32, tag="misc", bufs=2,
                                name=f"psv_{j}")
                for c in range(NCH):
                    nc.tensor.matmul(
                        psv,
                        xnT[:, c, j * 128 : (j + 1) * 128],
                        wv_t[:, c, :],
                        start=(c == 0),
                        stop=False,
                    )
                # += ones[s] x bv  (rank-1 bias update)
                nc.tensor.matmul(psv, vones_t, bv1_t, start=False, stop=True)
                return psv

            def emit_v_copy(j, psv, on_act=False):
                if on_act:
                    chained_act(
                        out=vpp[:, j, :, 0:HD],
                        in_=psv.rearrange("p (h v) -> p h v", v=HD),
                        func=mybir.ActivationFunctionType.Copy,
                    )
                else:
                    nc.vector.tensor_copy(
                        vpp[:, j, :, 0:HD],
                        psv.rearrange("p (h v) -> p h v", v=HD),
                    )

            # ================= attention unit (sb, p) =====================
            def emit_attn_unit(sb, p):
                q0 = sb * 512
                jmax = 4 * sb + 3
                yts = [
                    pscm.tile([HD + 1, 512], f32, tag="yt", bufs=2,
                              name=f"yt_{sb}_{p}_{hf}")
                    for hf in range(2)
                ]
                sts = {}
                pts = {}

                def emit_scores(j):
                    r = max(0, j - 4 * sb)
                    diag = j >= 4 * sb
                    L = 512 - 128 * r
                    st = pscm.tile([128, 1024], f32, tag="st", bufs=2,
                                   name=f"st_{sb}_{p}_{j}")
                    for hf in range(2):
                        rows = slice(hf * HD, (hf + 1) * HD)
                        # hf0 packs left in bank 0; hf1 must stay bank-aligned
                        # at 512 (matmul outputs cannot cross a PSUM bank)
                        lo = hf * 512
                        nc.tensor.matmul(
                            st[:, lo : lo + L],
                            kT[rows, p, j * 128 : (j + 1) * 128],
                            qT[rows, p, q0 + r * 128 : q0 + 512],
                            start=True, stop=not diag,
                        )
                    if diag:
                        # causal mask folded in on the PE:
                        # st[diag] += I.T @ maskT
                        for hf in range(2):
                            nc.tensor.matmul(
                                st[:, hf * 512 : hf * 512 + 128],
                                identb,
                                maskTb,
                                start=False, stop=True,
                            )
                    sts[j] = (st, L)

                def emit_exp(j):
                    st, L = sts.pop(j)
                    pt = ptp.tile([128, 1024], bf16, tag="pt")
                    # one wide exp across both heads (for r>0 the [L:512)
                    # strip is unread garbage)
                    chained_act(
                        out=pt[:, 0 : 512 + L],
                        in_=st[:, 0 : 512 + L],
                        func=mybir.ActivationFunctionType.Exp,
                    )
                    pts[j] = (pt, L)

                def emit_pv(j):
                    pt, L = pts.pop(j)
                    r = (512 - L) // 128
                    for hf in range(2):
                        nc.tensor.matmul(
                            yts[hf][:, r * 128 : 512],
                            vpp[:, j, 2 * p + hf, :],
                            pt[:, hf * 512 : hf * 512 + L],
                            start=(j == 0),
                            stop=(j == jmax),
                        )

                # software pipeline: scores(j+1) issued before pv(j) so the
                # in-order PE never head-of-line blocks on exp(j)
                emit_scores(0)
                for j in range(jmax + 1):
                    if j + 1 <= jmax:
                        emit_scores(j + 1)
                    emit_exp(j)
                    emit_pv(j)
                return yts

            # softmax normalization epilogue (row 64 of yts = sums); emitted
            # AFTER the next filler work so the DVE drains the PSUM-freeing
            # bias-adds/copies first
            def emit_epilogue(sb, p, yts):
                q0 = sb * 512
                for hf in range(2):
                    rows = slice(hf * HD, (hf + 1) * HD)
                    yt = yts[hf]
                    sinv = epip.tile([1, 512], f32, tag="sinv")
                    nc.vector.reciprocal(out=sinv, in_=yt[HD : HD + 1, :])
                    binv = epip.tile([HD, 512], f32, tag="binv")
                    nc.gpsimd.partition_broadcast(binv, sinv, channels=HD)
                    nc.vector.tensor_mul(
                        out=ytall[rows, p, q0 : q0 + 512],
                        in0=yt[0:HD, :],
                        in1=binv,
                    )

            # ================= output projection ==========================
            def emit_outproj(i):
                y_t = xpool.tile([128, 1024], f32, tag="y", bufs=2, name=f"y_{i}")
                for nh in range(2):
                    pso = pscm.tile([128, 512], f32, tag="misc", bufs=2,
                                    name=f"pso_{i}_{nh}")
                    for c in range(4):
                        nc.tensor.matmul(
                            pso,
                            ytall[:, c, i * 128 : (i + 1) * 128],
                            wout_t[:, c, nh * 512 : (nh + 1) * 512],
                            start=(c == 0),
                            stop=(c == 3),
                        )
                    nc.vector.tensor_copy(
                        y_t[:, nh * 512 : (nh + 1) * 512], pso
                    )
                    cdma(
                        out=out_d[i * 128 : (i + 1) * 128,
                                  nh * 512 : (nh + 1) * 512],
                        in_=y_t[:, nh * 512 : (nh + 1) * 512],
                    )

            # ================= emission schedule ==========================
            # DMA queue order: independent loads first (x blocks 0..7 and
            # the weight chunks needed earliest), dependent transposes after
            # their producers
            for i in range(8):
                emit_ln_load(i)
            load_wqk(0)
            load_wqk(4)
            for c in range(NCH):
                cdma(out=wv_t[:, c, :], in_=wv_d[c, :, :])
            cdma(out=bqk_t, in_=bqk_d[:, :, :])
            cdma(out=bv1_t, in_=bv1_d[:, :])
            cdma(out=vones_t, in_=vones_d[:, :])
            emit_ln_group(range(0, 4), pe=True)
            emit_ln_group(range(4, 8))
            for fb in (1, 5, 2, 6, 3, 7):
                load_wqk(fb)
            for j in range(4):
                emit_v_copy(j, emit_v(j))
            emit_qk(0, 0)
            for c in range(4):
                cdma(out=wout_t[:, c, :], in_=wout_d[c, :, :])

            for sb in range(NQS):
                for p in range(4):
                    if sb == 0 and p >= 1:
                        emit_qk(p, 0)
                    yts = emit_attn_unit(sb, p)
                    if sb < 3:
                        emit_qk(p, sb + 1, on_act=(sb <= 1))
                        emit_v_copy(4 * (sb + 1) + p,
                                    emit_v(4 * (sb + 1) + p),
                                    on_act=(sb <= 1))
                    if sb <= 1:
                        # stats early in the superblock; the Act-table-
                        # sensitive ln/exp ops + xh + transposes batched so
                        # xnT for the NEXT projections lands with slack
                        base = 8 + 4 * sb
                        if p <= 1:
                            emit_ln_stats(base + 2 * p)
                            emit_ln_stats(base + 2 * p + 1)
                        elif p == 2:
                            blocks = range(base, base + 4)
                            emit_ln_acts(blocks)
                            for i in blocks:
                                emit_ln_apply(i)
                            for i in blocks:
                                emit_ln_transpose(i)
                    emit_epilogue(sb, p, yts)
                    if sb >= 1:
                        emit_outproj(4 * (sb - 1) + p)
            for p in range(4):
                emit_outproj(12 + p)

            if DEBUG_DUMPS:
                cdma(out=dbg["xnT"][:, :, :], in_=xnT)
                cdma(out=dbg["qT"][:, :, :], in_=qT)
                cdma(out=dbg["kT"][:, :, :], in_=kT)
                cdma(out=dbg["vpp"][:, :, :, :], in_=vpp)
                cdma(out=dbg["ytall"][:, :, :], in_=ytall)

    nc.finalize()
    return nc


def _prep_core_inputs(x, ln_scale, ln_bias, w_qkv, b_qkv, w_out):
    """Host-side folding + per-core input maps."""
    scale = np.float32(HD ** -0.5)
    # qkv = xn@W + b_qkv, xn = z*ln_scale + ln_bias  =>  z @ (ln_scale*W) + (ln_bias@W + b_qkv)
    b_eff = b_qkv + np.einsum(
        "d,dhf->hf", ln_bias.astype(np.float64), w_qkv.astype(np.float64)
    ).astype(np.float32)
    w_eff = ln_scale[:, None, None] * w_qkv
    wq = w_eff[:, :, 0:64] * scale
    wk = w_eff[:, :, 64:128]
    wv = w_eff[:, :, 128:192]
    bq = b_eff[:, 0:64] * scale
    bk = b_eff[:, 64:128]
    bv = b_eff[:, 128:192]

    in_maps = []
    for core in range(8):
        b, g = core // 2, core % 2
        hsel = slice(g * HL, (g + 1) * HL)
        # [D, 4 pairs, 128] with head 2p in rows 0:64, head 2p+1 in 64:128
        qp = wq[:, hsel].reshape(D, 4, 128)
        kp = wk[:, hsel].reshape(D, 4, 128)
        wqk = np.concatenate(
            [qp.reshape(D, 512), kp.reshape(D, 512)], axis=1
        ).reshape(NCH, 128, 1024).astype(ml_dtypes.bfloat16)
        wv_g = np.ascontiguousarray(wv[:, hsel].reshape(D, 512)).reshape(
            NCH, 128, 512
        ).astype(ml_dtypes.bfloat16)
        bq_p = bq[hsel].reshape(4, 128)
        bk_p = bk[hsel].reshape(4, 128)
        bqk = np.ascontiguousarray(
            np.stack([bq_p, bk_p], axis=0).transpose(2, 0, 1)
        )
        bv1 = np.ascontiguousarray(bv[hsel].reshape(1, 512))
        wout = np.ascontiguousarray(
            w_out[g * 512 : (g + 1) * 512, :].reshape(4, 128, 1024)
        ).astype(ml_dtypes.bfloat16)
        in_maps.append(
            {
                "x": np.ascontiguousarray(x[b]),
                "wqk": np.ascontiguousarray(wqk),
                "wv": wv_g,
                "bqk": bqk,
                "bv1": bv1,
                "vones": np.ones((1, 128), np.float32),
                "wout": wout,
            }
        )
    return in_maps


def kernel(x, mask, ln_scale, ln_bias, w_qkv, b_qkv, w_out, b_out, **run_kwargs):
    x = np.asarray(x, np.float32)
    ln_scale = np.asarray(ln_scale, np.float32)
    ln_bias = np.asarray(ln_bias, np.float32)
    w_qkv = np.asarray(w_qkv, np.float32)
    b_qkv = np.asarray(b_qkv, np.float32)
    w_out = np.asarray(w_out, np.float32)
    b_out = np.asarray(b_out, np.float32)
    if "nc" not in _cache:
        _cache["nc"] = build_program()
    nc = _cache["nc"]
    in_maps = _prep_core_inputs(x, ln_scale, ln_bias, w_qkv, b_qkv, w_out)
    res = run_bass_kernel_spmd(nc, in_maps, list(range(8)), **run_kwargs)
    _cache["last_result"] = res
    out = np.empty((B, S, D), np.float32)
    for b in range(B):
        out[b] = res.results[2 * b]["out"] + res.results[2 * b + 1]["out"]
    out += np.asarray(b_out)[None, None, :]
    return out


# revision 39
# speedup vs baseline: 1.3087x; 1.0044x over previous
"""Causal self-attention block (LN -> QKV -> causal attention -> out-proj)
on 8 Trainium2 NeuronCores.

Sharding: core = 2*batch + head_group. Each core handles one batch element
(S=2048 tokens) and 8 of the 16 heads (tensor-parallel split of w_qkv along
the head axis and w_out along its input dim). The two partial outputs per
batch are summed on the host (the all-reduce of the sharding hint).

Device kernel layout strategy (per core):
  - LayerNorm stats on DVE; rstd computed as exp(-0.5*ln(var+eps)) on the
    Activation engine so the whole kernel needs only ONE act table
    (natural_log_exp: ln+exp) -- no table thrash between LN and softmax.
  - Normalized x is cast to bf16 and transposed by the DMA xbar
    (dma_start_transpose), freeing the PE of all transposes.
  - QKV projection computes q^T/k^T in [head_dim, s] layout directly and V in
    natural [s, head_dim] layout, so causal attention needs no further
    transposes: scores are computed transposed, ST[k, q] = k . q, softmax'd
    along the partition-free axis via exp + a ones-column appended to V
    (the PV matmul then yields both y^T and the softmax row-sums).
  - Emission order interleaves QKV projection / LN of later blocks / output
    projection INTO the attention superblock loop so the in-order PE stream
    always has matmul work while the Activation engine runs softmax exps.
  - Softmax normalization: row-sums -> reciprocal (DVE) -> broadcast across
    partitions on the Pool engine (partition_broadcast) -> DVE multiply.
  - Output projection streams straight from PSUM to DRAM via DMA.
  - ln_scale/ln_bias/b_qkv/softmax-scale are folded into weights on host.
"""

import os

# the device path runs through jax's axon PJRT plugin; make sure a
# pre-set JAX_PLATFORMS doesn't hide it (unset = all plugins load)
_jp = os.environ.get("JAX_PLATFORMS")
if _jp and "axon" not in _jp:
    os.environ["JAX_PLATFORMS"] = f"axon,{_jp}"

import ml_dtypes
import numpy as np

import concourse.bass as bass
import concourse.mybir as mybir
import concourse.tile as tile
from concourse import bacc
from concourse.bass_utils import run_bass_kernel_spmd
from concourse.masks import make_identity

B, S, D, H, HD = 4, 2048, 1024, 16, 64
HL = H // 2          # heads per core (local)
NCH = D // 128       # 8 contraction chunks
NSB = S // 128       # 16 s-blocks
NQS = S // 512       # 4 q-superblocks
NEG = -1.0e38
LN_EPS = 1e-6

f32 = mybir.dt.float32
f32r = mybir.dt.float32r
bf16 = mybir.dt.bfloat16

_cache = {}

DEBUG_DUMPS = False


def build_program():
    nc = bacc.Bacc()

    x_d = nc.declare_dram_parameter("x", [S, D], f32, isOutput=False)
    wqk_d = nc.declare_dram_parameter("wqk", [NCH, 128, 1024], bf16, isOutput=False)
    wv_d = nc.declare_dram_parameter("wv", [NCH, 128, 512], bf16, isOutput=False)
    bqk_d = nc.declare_dram_parameter("bqk", [128, 2, 4], f32, isOutput=False)
    bv1_d = nc.declare_dram_parameter("bv1", [1, 512], f32r, isOutput=False)
    vones_d = nc.declare_dram_parameter("vones", [1, 128], f32r, isOutput=False)
    wout_d = nc.declare_dram_parameter("wout", [4, 128, 1024], bf16, isOutput=False)
    out_d = nc.declare_dram_parameter("out", [S, D], f32, isOutput=True)
    if DEBUG_DUMPS:
        dbg = {
            "xnT": nc.declare_dram_parameter("d_xnT", [128, NCH, S], bf16, isOutput=True),
            "qT": nc.declare_dram_parameter("d_qT", [128, 4, S], bf16, isOutput=True),
            "kT": nc.declare_dram_parameter("d_kT", [128, 4, S], bf16, isOutput=True),
            "vpp": nc.declare_dram_parameter("d_vpp", [128, NSB, HL, HD + 1], bf16, isOutput=True),
            "ytall": nc.declare_dram_parameter("d_ytall", [128, 4, S], bf16, isOutput=True),
            "xh": nc.declare_dram_parameter("d_xh", [4, 128, D], bf16, isOutput=True),
            "rstd": nc.declare_dram_parameter("d_rstd", [4, 128, 1], f32, isOutput=True),
        }

    with tile.TileContext(nc, pool_alloc_mode="queue") as tc:
        with (
            tc.tile_pool(name="singles", bufs=1) as singles,
            tc.tile_pool(name="qkT", bufs=1) as qkTp,
            tc.tile_pool(name="vpool", bufs=1) as vpool,
            tc.tile_pool(name="xnTp", bufs=1) as xnTp,
            tc.tile_pool(name="ytallp", bufs=1) as ytallp,
            tc.tile_pool(name="xpool", bufs=3) as xpool,
            tc.tile_pool(name="spool", bufs=8) as spool,
            tc.tile_pool(name="ptp", bufs=4) as ptp,
            tc.tile_pool(name="epi", bufs=2) as epip,
            tc.tile_pool(name="dstage", bufs=4, space="DRAM") as dstage,
            tc.tile_pool(name="pscm", bufs=1, space="PSUM") as pscm,
        ):
            # chain every DMA so the single in-order DMA FIFO processes in
            # exactly emission order: xbar-mode transitions (copy<->transpose
            # serialize on full drain) then only hit group boundaries we chose
            dma_chain = [None]

            def chained_dma(inst):
                if dma_chain[0] is not None:
                    bass._add_dep_helper(
                        inst.ins, dma_chain[0].ins, sync=False,
                        reason="dma fifo order",
                    )
                dma_chain[0] = inst
                return inst

            def cdma(out, in_):
                return chained_dma(nc.sync.dma_start(out=out, in_=in_))

            def cdma_t(out, in_):
                return chained_dma(nc.sync.dma_start_transpose(out=out, in_=in_))

            # ---- constants ----
            identb = singles.tile([128, 128], bf16)
            make_identity(nc, identb)
            identf = singles.tile([128, 128], f32)
            make_identity(nc, identf)
            maskTb = singles.tile([128, 128], bf16)
            nc.gpsimd.memset(maskTb, 0.0)
            nc.gpsimd.affine_select(
                out=maskTb, in_=maskTb,
                compare_op=mybir.AluOpType.is_ge,
                fill=NEG, base=0,
                pattern=[[1, 128]], channel_multiplier=-1,
            )
            eps_t = singles.tile([128, 1], f32)
            nc.vector.memset(eps_t, LN_EPS)
            bqk_t = singles.tile([128, 2, 4], f32)
            bv1_t = singles.tile([1, 512], f32r)
            vones_t = singles.tile([1, 128], f32r)

            # ---- weights (all resident in SBUF; bf16) ----
            # the sim models ONE in-order DMA FIFO with head-of-line
            # blocking, so DMA emission order is scheduling: independent
            # loads (x blocks, weight chunks) go first, dependent DMAs
            # (xbar transposes of xh) are emitted right after their
            # producers so the queue never stalls long on them
            wqk_t = singles.tile([128, NCH, 1024], bf16)
            wv_t = singles.tile([128, NCH, 512], bf16)
            wout_t = singles.tile([128, 4, 1024], bf16)

            def load_wqk(fb):
                cdma(
                    out=wqk_t[:, :, fb * 128 : (fb + 1) * 128],
                    in_=wqk_d[:, :, fb * 128 : (fb + 1) * 128].rearrange(
                        "c d f -> d c f"
                    ),
                )

            # ---- persistent activations ----
            qT = qkTp.tile([128, 4, S], bf16)   # [pair-row, pair, s]
            kT = qkTp.tile([128, 4, S], bf16)
            # V'' [s-row, s-block, head, 65] (col 64 = ones)
            vpp = vpool.tile([128, NSB, HL, HD + 1], bf16)
            nc.gpsimd.memset(vpp[:, :, :, HD : HD + 1], 1.0)
            xnT = xnTp.tile([128, NCH, S], bf16)
            ytall = ytallp.tile([128, 4, S], bf16)  # [pair-row, pair, s]

            # ================= per-block LN + DMA-xbar transpose ==========
            # rstd = exp(-0.5 * ln(var + eps)).  Ln and Exp live in different
            # greedy act tables, so the Ln/Exp ops are BATCHED per 4-block
            # group to bound table reloads (2 per group).  Every Activation-
            # engine op is chained with a nosync dep so the tile scheduler
            # cannot interleave Ln/Exp runs (which would thrash tables).
            ln_state = {}
            act_chain = [None]

            def chained_act(**kw):
                inst = nc.scalar.activation(**kw)
                if act_chain[0] is not None:
                    bass._add_dep_helper(
                        inst.ins, act_chain[0].ins, sync=False,
                        reason="act table batching order",
                    )
                act_chain[0] = inst
                return inst

            def emit_ln_load(i):
                # two half-loads so bn_stats can chase the DMA
                x_t = xpool.tile([128, D], f32, tag="x", bufs=6, name=f"x_{i}")
                cdma(out=x_t[:, 0:512], in_=x_d[i * 128 : (i + 1) * 128, 0:512])
                cdma(out=x_t[:, 512:1024],
                     in_=x_d[i * 128 : (i + 1) * 128, 512:1024])
                ln_state[i] = (x_t,)

            def emit_ln_stats(i):
                if i not in ln_state:
                    emit_ln_load(i)
                (x_t,) = ln_state[i]
                stats = spool.tile([128, 2, 6], f32, tag="stats")
                nc.vector.bn_stats(out=stats[:, 0, :], in_=x_t[:, 0:512])
                nc.vector.bn_stats(out=stats[:, 1, :], in_=x_t[:, 512:1024])
                mv = spool.tile([128, 2], f32, tag="mv", name=f"mv_{i}")
                nc.vector.bn_aggr(out=mv, in_=stats)
                ln_state[i] = (x_t, mv)

            def emit_ln_acts(blocks):
                lnvs = {}
                for i in blocks:
                    lnv = spool.tile([128, 1], f32, tag="lnv",
                                     name=f"lnv_{i}")
                    chained_act(
                        out=lnv, in_=ln_state[i][1][:, 1:2],
                        func=mybir.ActivationFunctionType.Ln,
                        bias=eps_t, scale=1.0,
                    )
                    lnvs[i] = lnv
                for i in blocks:
                    rstd = spool.tile([128, 1], f32, tag="rstd",
                                      name=f"rstd_{i}")
                    chained_act(
                        out=rstd, in_=lnvs[i],
                        func=mybir.ActivationFunctionType.Exp,
                        bias=0.0, scale=-0.5,
                    )
                    ln_state[i] = ln_state[i] + (rstd,)

            def emit_ln_apply(i, pe=False):
                x_t, mv, rstd = ln_state.pop(i)
                if pe:
                    # PE-transpose path for the first blocks: the PE is idle
                    # during the prologue and this also warms its pstate
                    xh = xpool.tile([128, D], f32, tag="xhf", bufs=4,
                                    name=f"xhf_{i}")
                else:
                    xh = xpool.tile([128, D], bf16, tag="xh", bufs=3,
                                    name=f"xh_{i}")
                nc.vector.tensor_scalar(
                    out=xh, in0=x_t,
                    scalar1=mv[:, 0:1], scalar2=rstd,
                    op0=mybir.AluOpType.subtract, op1=mybir.AluOpType.mult,
                )
                if pe:
                    for g in range(2):
                        pst = pscm.tile([128, 4, 128], f32, tag="misc",
                                        bufs=2, name=f"pst_{i}_{g}")
                        for k in range(4):
                            c = 4 * g + k
                            nc.tensor.transpose(
                                pst[:, k, :],
                                xh[:, c * 128 : (c + 1) * 128],
                                identf,
                            )
                        chained_act(
                            out=xnT[:, 4 * g : 4 * g + 4,
                                    i * 128 : (i + 1) * 128],
                            in_=pst,
                            func=mybir.ActivationFunctionType.Copy,
                        )
                    return
                # stage xh through DRAM: the compiled DmaTransposeAnt path
                # does not reliably honor engine-write semaphores on its SBUF
                # source, but a DRAM source written by an ordinary DMACopy
                # (sem-correct) on the same in-order FIFO is safe
                xhd = dstage.tile([128, D], bf16, tag="xhd", name=f"xhd_{i}")
                cdma(out=xhd, in_=xh)
                ln_state[i] = xhd

            def emit_ln_transpose(i):
                xhd = ln_state.pop(i)
                cdma_t(
                    out=xnT[:, :, i * 128 : (i + 1) * 128], in_=xhd
                )

            def emit_ln_group(blocks, pe=False):
                for i in blocks:
                    emit_ln_stats(i)
                emit_ln_acts(blocks)
                for i in blocks:
                    emit_ln_apply(i, pe=pe)
                if not pe:
                    for i in blocks:
                        emit_ln_transpose(i)

            # ================= QKV projection pieces ======================
            def emit_qk(p, sb, on_act=False):
                for t, dest in ((0, qT), (1, kT)):
                    fb = t * 4 + p
                    ps = pscm.tile([128, 512], f32, tag="misc", bufs=2,
                                   name=f"qkps_{t}_{p}_{sb}")
                    for c in range(NCH):
                        nc.tensor.matmul(
                            ps,
                            wqk_t[:, c, fb * 128 : (fb + 1) * 128],
                            xnT[:, c, sb * 512 : (sb + 1) * 512],
                            start=(c == 0),
                            stop=(c == NCH - 1),
                        )
                    if on_act:
                        # bias-add as Identity activation: Identity is in
                        # every act table, and the Act engine idles while the
                        # DVE saturates in these stretches
                        chained_act(
                            out=dest[:, p, sb * 512 : (sb + 1) * 512],
                            in_=ps,
                            func=mybir.ActivationFunctionType.Identity,
                            bias=bqk_t[:, t, p : p + 1], scale=1.0,
                        )
                    else:
                        nc.vector.tensor_scalar_add(
                            out=dest[:, p, sb * 512 : (sb + 1) * 512],
                            in0=ps,
                            scalar1=bqk_t[:, t, p : p + 1],
                        )

            def emit_v(j):
                psv = pscm.tile([128, 512], f32, tag="misc", bufs=2,
                                name=f"psv_{j}")
                for c in range(NCH):
                    nc.tensor.matmul(
                        psv,
                        xnT[:, c, j * 128 : (j + 1) * 128],
                        wv_t[:, c, :],
                        start=(c == 0),
                        stop=False,
                    )
                # += ones[s] x bv  (rank-1 bias update)
                nc.tensor.matmul(psv, vones_t, bv1_t, start=False, stop=True)
                return psv

            def emit_v_copy(j, psv, on_act=False):
                if on_act:
                    chained_act(
                        out=vpp[:, j, :, 0:HD],
                        in_=psv.rearrange("p (h v) -> p h v", v=HD),
                        func=mybir.ActivationFunctionType.Copy,
                    )
                else:
                    nc.vector.tensor_copy(
                        vpp[:, j, :, 0:HD],
                        psv.rearrange("p (h v) -> p h v", v=HD),
                    )

            # ================= attention unit (sb, p) =====================
            def emit_attn_unit(sb, p):
                q0 = sb * 512
                jmax = 4 * sb + 3
                yts = [
                    pscm.tile([HD + 1, 512], f32, tag="yt", bufs=2,
                              name=f"yt_{sb}_{p}_{hf}")
                    for hf in range(2)
                ]
                sts = {}
                pts = {}

                def emit_scores(j):
                    r = max(0, j - 4 * sb)
                    diag = j >= 4 * sb
                    L = 512 - 128 * r
                    st = pscm.tile([128, 1024], f32, tag="st", bufs=2,
                                   name=f"st_{sb}_{p}_{j}")
                    for hf in range(2):
                        rows = slice(hf * HD, (hf + 1) * HD)
                        # hf0 packs left in bank 0; hf1 must stay bank-aligned
                        # at 512 (matmul outputs cannot cross a PSUM bank)
                        lo = hf * 512
                        nc.tensor.matmul(
                            st[:, lo : lo + L],
                            kT[rows, p, j * 128 : (j + 1) * 128],
                            qT[rows, p, q0 + r * 128 : q0 + 512],
                            start=True, stop=not diag,
                        )
                    if diag:
                        # causal mask folded in on the PE:
                        # st[diag] += I.T @ maskT
                        for hf in range(2):
                            nc.tensor.matmul(
                                st[:, hf * 512 : hf * 512 + 128],
                                identb,
                                maskTb,
                                start=False, stop=True,
                            )
                    sts[j] = (st, L)

                def emit_exp(j):
                    st, L = sts.pop(j)
                    pt = ptp.tile([128, 1024], bf16, tag="pt")
                    # one wide exp across both heads (for r>0 the [L:512)
                    # strip is unread garbage)
                    chained_act(
                        out=pt[:, 0 : 512 + L],
                        in_=st[:, 0 : 512 + L],
                        func=mybir.ActivationFunctionType.Exp,
                    )
                    pts[j] = (pt, L)

                def emit_pv(j):
                    pt, L = pts.pop(j)
                    r = (512 - L) // 128
                    for hf in range(2):
                        nc.tensor.matmul(
                            yts[hf][:, r * 128 : 512],
                            vpp[:, j, 2 * p + hf, :],
                            pt[:, hf * 512 : hf * 512 + L],
                            start=(j == 0),
                            stop=(j == jmax),
                        )

                # software pipeline: scores(j+1) issued before pv(j) so the
                # in-order PE never head-of-line blocks on exp(j)
                emit_scores(0)
                for j in range(jmax + 1):
                    if j + 1 <= jmax:
                        emit_scores(j + 1)
                    emit_exp(j)
                    emit_pv(j)
                return yts

            # softmax normalization epilogue (row 64 of yts = sums); emitted
            # AFTER the next filler work so the DVE drains the PSUM-freeing
            # bias-adds/copies first
            def emit_epilogue(sb, p, yts):
                q0 = sb * 512
                for hf in range(2):
                    rows = slice(hf * HD, (hf + 1) * HD)
                    yt = yts[hf]
                    sinv = epip.tile([1, 512], f32, tag="sinv")
                    nc.vector.reciprocal(out=sinv, in_=yt[HD : HD + 1, :])
                    binv = epip.tile([HD, 512], f32, tag="binv")
                    nc.gpsimd.partition_broadcast(binv, sinv, channels=HD)
                    nc.vector.tensor_mul(
                        out=ytall[rows, p, q0 : q0 + 512],
                        in0=yt[0:HD, :],
                        in1=binv,
                    )

            # ================= output projection ==========================
            def emit_outproj(i):
                y_t = xpool.tile([128, 1024], f32, tag="y", bufs=2, name=f"y_{i}")
                for nh in range(2):
                    pso = pscm.tile([128, 512], f32, tag="misc", bufs=2,
                                    name=f"pso_{i}_{nh}")
                    for c in range(4):
                        nc.tensor.matmul(
                            pso,
                            ytall[:, c, i * 128 : (i + 1) * 128],
                            wout_t[:, c, nh * 512 : (nh + 1) * 512],
                            start=(c == 0),
                            stop=(c == 3),
                        )
                    nc.vector.tensor_copy(
                        y_t[:, nh * 512 : (nh + 1) * 512], pso
                    )
                    cdma(
                        out=out_d[i * 128 : (i + 1) * 128,
                                  nh * 512 : (nh + 1) * 512],
                        in_=y_t[:, nh * 512 : (nh + 1) * 512],
                    )

            # ================= emission schedule ==========================
            # DMA queue order: independent loads first (x blocks 0..7 and
            # the weight chunks needed earliest), dependent transposes after
            # their producers
            for i in range(8):
                emit_ln_load(i)
            load_wqk(0)
            load_wqk(4)
            for c in range(NCH):
                cdma(out=wv_t[:, c, :], in_=wv_d[c, :, :])
            cdma(out=bqk_t, in_=bqk_d[:, :, :])
            cdma(out=bv1_t, in_=bv1_d[:, :])
            cdma(out=vones_t, in_=vones_d[:, :])
            emit_ln_group(range(0, 4), pe=True)
            emit_ln_group(range(4, 8))
            for fb in (1, 5, 2, 6, 3, 7):
                load_wqk(fb)
            for j in range(4):
                emit_v_copy(j, emit_v(j))
            emit_qk(0, 0)
            for c in range(4):
                cdma(out=wout_t[:, c, :], in_=wout_d[c, :, :])

            for sb in range(NQS):
                for p in range(4):
                    if sb == 0 and p >= 1:
                        emit_qk(p, 0)
                    yts = emit_attn_unit(sb, p)
                    if sb < 3:
                        emit_qk(p, sb + 1, on_act=(sb <= 1))
                        emit_v_copy(4 * (sb + 1) + p,
                                    emit_v(4 * (sb + 1) + p),
                                    on_act=(sb <= 1))
                    if sb <= 1:
                        # stats early in the superblock; the Act-table-
                        # sensitive ln/exp ops + xh + transposes batched so
                        # xnT for the NEXT projections lands with slack
                        base = 8 + 4 * sb
                        if p <= 1:
                            emit_ln_stats(base + 2 * p)
                            emit_ln_stats(base + 2 * p + 1)
                        elif p == 2:
                            blocks = range(base, base + 4)
                            emit_ln_acts(blocks)
                            for i in blocks:
                                emit_ln_apply(i)
                            for i in blocks:
                                emit_ln_transpose(i)
                    emit_epilogue(sb, p, yts)
                    if sb >= 1:
                        emit_outproj(4 * (sb - 1) + p)
            for p in range(4):
                emit_outproj(12 + p)

            if DEBUG_DUMPS:
                cdma(out=dbg["xnT"][:, :, :], in_=xnT)
                cdma(out=dbg["qT"][:, :, :], in_=qT)
                cdma(out=dbg["kT"][:, :, :], in_=kT)
                cdma(out=dbg["vpp"][:, :, :, :], in_=vpp)
                cdma(out=dbg["ytall"][:, :, :], in_=ytall)

    nc.finalize()
    return nc


def _prep_core_inputs(x, ln_scale, ln_bias, w_qkv, b_qkv, w_out):
    """Host-side folding + per-core input maps."""
    scale = np.float32(HD ** -0.5)
    # qkv = xn@W + b_qkv, xn = z*ln_scale + ln_bias  =>  z @ (ln_scale*W) + (ln_bias@W + b_qkv)
    b_eff = b_qkv + np.einsum(
        "d,dhf->hf", ln_bias.astype(np.float64), w_qkv.astype(np.float64)
    ).astype(np.float32)
    w_eff = ln_scale[:, None, None] * w_qkv
    wq = w_eff[:, :, 0:64] * scale
    wk = w_eff[:, :, 64:128]
    wv = w_eff[:, :, 128:192]
    bq = b_eff[:, 0:64] * scale
    bk = b_eff[:, 64:128]
    bv = b_eff[:, 128:192]

    in_maps = []
    for core in range(8):
        b, g = core // 2, core % 2
        hsel = slice(g * HL, (g + 1) * HL)
        # [D, 4 pairs, 128] with head 2p in rows 0:64, head 2p+1 in 64:128
        qp = wq[:, hsel].reshape(D, 4, 128)
        kp = wk[:, hsel].reshape(D, 4, 128)
        wqk = np.concatenate(
            [qp.reshape(D, 512), kp.reshape(D, 512)], axis=1
        ).reshape(NCH, 128, 1024).astype(ml_dtypes.bfloat16)
        wv_g = np.ascontiguousarray(wv[:, hsel].reshape(D, 512)).reshape(
            NCH, 128, 512
        ).astype(ml_dtypes.bfloat16)
        bq_p = bq[hsel].reshape(4, 128)
        bk_p = bk[hsel].reshape(4, 128)
        bqk = np.ascontiguousarray(
            np.stack([bq_p, bk_p], axis=0).transpose(2, 0, 1)
        )
        bv1 = np.ascontiguousarray(bv[hsel].reshape(1, 512))
        wout = np.ascontiguousarray(
            w_out[g * 512 : (g + 1) * 512, :].reshape(4, 128, 1024)
        ).astype(ml_dtypes.bfloat16)
        in_maps.append(
            {
                "x": np.ascontiguousarray(x[b]),
                "wqk": np.ascontiguousarray(wqk),
                "wv": wv_g,
                "bqk": bqk,
                "bv1": bv1,
                "vones": np.ones((1, 128), np.float32),
                "wout": wout,
            }
        )
    return in_maps


def kernel(x, mask, ln_scale, ln_bias, w_qkv, b_qkv, w_out, b_out, **run_kwargs):
    x = np.asarray(x, np.float32)
    ln_scale = np.asarray(ln_scale, np.float32)
    ln_bias = np.asarray(ln_bias, np.float32)
    w_qkv = np.asarray(w_qkv, np.float32)
    b_qkv = np.asarray(b_qkv, np.float32)
    w_out = np.asarray(w_out, np.float32)
    b_out = np.asarray(b_out, np.float32)
    if "nc" not in _cache:
        _cache["nc"] = build_program()
    nc = _cache["nc"]
    in_maps = _prep_core_inputs(x, ln_scale, ln_bias, w_qkv, b_qkv, w_out)
    res = run_bass_kernel_spmd(nc, in_maps, list(range(8)), **run_kwargs)
    _cache["last_result"] = res
    out = np.empty((B, S, D), np.float32)
    for b in range(B):
        out[b] = res.results[2 * b]["out"] + res.results[2 * b + 1]["out"]
    out += np.asarray(b_out)[None, None, :]
    return out
